# revision 50
# baseline (speedup 1.0000x reference)
"""Trainium2 Bass kernel for nn_AttnBlock (block-causal single-head attention
over video tokens, with RMS-norm and 1x1-conv q/k/v/out projections).

Shapes: x [2, 512, 8, 32, 32] -> S = 8*1024 = 8192 tokens per batch,
block-causal over frames (1024 tokens per frame).

Sharding: core = 4*b + ch handles batch b and the ch-th 256-query chunk of
EVERY frame -> all 8 cores run an identical instruction stream (SPMD) with
perfectly balanced block-causal attention work.

Key algebraic restructure (vs a direct port of the reference):
  * The 1x1 convs commute with attention, so the K and V projections are
    ELIMINATED by composing weights on the host:
      scores = K^T Q = x^T (Wk^T Wq) x       -> one matrix M, applied only
                                                to the 2048 query tokens/core
      P V    = Wv (P x)                      -> attention runs in x-space
      Wo(Wv (P x)) = (Wo Wv)(P x)            -> one composed output proj
  * The RMS-norm scale r[t] = sqrt(C)/||x_t|| commutes past the linear maps:
      - r_q is folded into the T = M x_q build (row broadcast, tiny)
      - r_k enters via the ACT exp's per-partition scale AND bias APs:
          et = r_k * exp(c * r_k * s_raw)    (bias = ln r_k)
        so PV over raw x^T tiles yields o~ = sum_k p_k r_k x_k directly
      - the denominator matmul uses lhsT = fp8(1/r_k) so den = sum_k exp
  * bk drops exactly (softmax per-query shift invariance); bq folds into the
    T bias (exact for bq=0, first-order otherwise); bv,bo fold into the
    residual (exact, since softmax rows sum to 1).

fp8: x itself is shipped as fp8e4 in BOTH layouts (channel-partition for
score lhsT / T rhs, token-partition for PV lhsT). M is fp8 scaled x128;
WoWv stays bf16. All big matmuls are fp8 DoubleRow (K=256 per column).
"""

import numpy as np
import ml_dtypes
from contextlib import ExitStack

# ---------------------------------------------------------------------------
# Walrus workaround: this container's walrus build accepts at most ONE sync
# wait command per instruction. Split excess waits onto same-engine NOPs
# (waits execute strictly earlier -> safe), including the Tile exit drain.
# ---------------------------------------------------------------------------
import bass_rust
import concourse.bass as bass
import concourse.mybir as mybir
import concourse.tile as tile
from concourse.vector_clock import ScopedClock
from concourse.bass_utils import run_bass_kernel_spmd

_MAX_WAITS = 1
_orig_lower = tile.TileContext._lower_ordered_insts


def _split_waits(nc, ordered):
    for bb, insts in ordered.items():
        out = []
        for inst in insts:
            si = inst.sync_info
            waits = list(si.on_wait) if si is not None and si.on_wait else []
            if (
                len(waits) > _MAX_WAITS
                and inst.engine is not None
                and inst.engine != mybir.EngineType.Unassigned
            ):
                for w in waits[:-_MAX_WAITS]:
                    out.append(
                        mybir.InstNoOp(
                            name=nc.get_next_instruction_name(),
                            engine=inst.engine,
                            bass_nofuse=True,
                            sync_info=mybir.SyncInfo(on_wait=[w], on_update=[]),
                        )
                    )
                si.on_wait = waits[-_MAX_WAITS:]
            out.append(inst)
        ordered[bb] = out


def _patched_lower(self, ordered):
    _split_waits(self.nc, ordered)
    return _orig_lower(self, ordered)


def _patched_drain_and_barrier(self, tick_clock, wait_clock):
    nc = self.nc
    drain_inst = nc.sync.drain()
    wait_clock.add_sem_waits(
        drain_inst.ins, ScopedClock({None: tick_clock.global_clock})
    )
    si = drain_inst.ins.sync_info
    waits = list(si.on_wait or []) if si is not None else []
    if len(waits) > _MAX_WAITS:
        si.on_wait = waits[:_MAX_WAITS]
        # distribute the remaining waits across all engine queues so the
        # exit drain's wait-processing runs in parallel instead of as one
        # serial NOP chain on the sync sequencer
        engines = [nc.sync, nc.vector, nc.scalar, nc.tensor, nc.gpsimd]
        for k, i in enumerate(range(_MAX_WAITS, len(waits), _MAX_WAITS)):
            n = engines[k % len(engines)].nop(nofuse=True)
            n.ins.sync_info = bass_rust.SyncInfo(
                on_wait=waits[i:i + _MAX_WAITS], on_update=[]
            )
    nc.all_engine_barrier()
    assert self.sems is not None
    popped = nc._tile_sem_poison_stack.pop()
    assert popped is self._sem_poison
    nc.clear_and_free_semaphores(list(self.sems.allocated().values()))
    nc.all_engine_barrier()


def _install_fix():
    tile.TileContext._lower_ordered_insts = _patched_lower
    tile.TileContext._drain_and_barrier = _patched_drain_and_barrier


# ---------------------------------------------------------------------------
# Problem constants (hardcoded per contract)
# ---------------------------------------------------------------------------
B, C, F, H, W = 2, 512, 8, 32, 32
HW = H * W            # 1024 tokens per frame
S = F * HW            # 8192 tokens per batch
P = 128
CT = C // P           # 4 channel tiles
QB = 256              # query block per frame per core
TQ = F * QB           # 2048 queries per core
CH = 512              # phase-A token chunk
NCH = S // CH         # 16
NKT = S // P          # 64 key tiles of 128
KP = NKT // 2         # 32 key-tile pairs
N_CORES = 8

WS_M = 128.0          # host-side fp8 range scale on M = Wk^T Wq
WS_O = 128.0          # host-side fp8 range scale on WoWv

f32 = mybir.dt.float32
bf16 = mybir.dt.bfloat16
fp8 = mybir.dt.float8e4
AF = mybir.ActivationFunctionType
DR = mybir.MatmulPerfMode.DoubleRow
ALU = None


def _build_nc(reps=1):
    nc = bass.Bass("TRN2")

    xk8 = nc.dram_tensor("xk8", [P, CT, S], fp8, kind="ExternalInput")
    xt8 = nc.dram_tensor("xt8", [P, KP, 2, C], fp8, kind="ExternalInput")
    xqres = nc.dram_tensor("xqres", [P, CT, F, QB], f32, kind="ExternalInput")
    m_t = nc.dram_tensor("m_t", [P, CT, C], fp8, kind="ExternalInput")
    wow_t = nc.dram_tensor("wow_t", [P, CT, C], fp8, kind="ExternalInput")
    bq_col = nc.dram_tensor("bq_col", [P, CT], f32, kind="ExternalInput")
    out = nc.dram_tensor("out", [P, CT, F, QB], f32, kind="ExternalOutput")

    with tile.TileContext(nc) as tc:
        for _ in range(reps):
            _emit_once(nc, tc, xk8, xt8, xqres, m_t, wow_t, bq_col, out)
    return nc


def _emit_once(nc, tc, xk8, xt8, xqres, m_t, wow_t, bq_col, out):
    import concourse.alu_op_type as alu
    with ExitStack() as ctx:
        big = ctx.enter_context(tc.tile_pool(name="big", bufs=1))
        xk_sb = big.tile([P, CT, S], fp8)
        xt_sb = big.tile([P, KP, 2, C], fp8)
        T_sb = big.tile([P, CT, TQ], fp8)
        sccol = big.tile([P, NKT], f32)     # exp per-partition scale c*r_k
        bcol = big.tile([P, NKT], f32)      # exp per-partition bias ln(r_k)
        # 1/r_k denominator lhsT, padded to stride 16 so the DR LDWEIGHTS
        # AP matches the encodable pattern (row stride >= 16)
        ir8 = big.tile([P, NKT, 16], fp8)

        const = ctx.enter_context(tc.tile_pool(name="const", bufs=1))
        ones_f8 = const.tile([P, 2, 16], fp8)
        nc.vector.memset(ones_f8, 1.0)
        ones_col_bf = const.tile([1, P], bf16)
        nc.vector.memset(ones_col_bf, 1.0)
        lnc_sb = const.tile([P, 1], f32)
        m_sb = const.tile([P, CT, C], fp8)
        nc.sync.dma_start(out=m_sb, in_=m_t[:, :, :])
        wow_sb = const.tile([P, CT, C], fp8)
        nc.sync.dma_start(out=wow_sb, in_=wow_t[:, :, :])
        bqc_sb = const.tile([P, CT], f32)
        nc.sync.dma_start(out=bqc_sb, in_=bq_col[:, :])

        # exp scale constant: exp arg = (r_k / (sqrt(C)*WS_M)) * ps
        c_const = 1.0 / (float(np.sqrt(C)) * WS_M)
        nc.vector.memset(lnc_sb, float(np.log(c_const)))

        with (
            tc.tile_pool(name="sq", bufs=3) as sqp,
            tc.tile_pool(name="lnr", bufs=5) as lnp,
            tc.tile_pool(name="lnd", bufs=5, space="DRAM") as lndp,
            tc.tile_pool(name="rbq", bufs=2) as rbqp,
            tc.tile_pool(name="xres", bufs=2) as xrp,
            tc.tile_pool(name="etp", bufs=8) as etp,
            tc.tile_pool(name="smp", bufs=2) as smp,
            tc.tile_pool(name="outp", bufs=3) as outp,
            tc.tile_pool(name="ps", bufs=3, space="PSUM") as psbs,
            tc.tile_pool(name="psbo", bufs=1, space="PSUM") as psbo,
        ):
            # --------------------------------------------------------------
            # Phase A chunk: load x chunk (both layouts), sumsq -> ln ->
            # (transpose to columns) -> exp scale/bias/inv_r; on even chunks
            # build T = (M x_q + wkbq) * r_q for the frame's query block.
            # --------------------------------------------------------------
            chunk_state = {}

            def emit_chunk_a(ck):
                # DMA both layouts, square (split Pool/DVE), row-sumsq
                # matmuls, ln, DRAM bounce of the ln row
                sl = slice(ck * CH, (ck + 1) * CH)
                nc.sync.dma_start(out=xk_sb[:, :, sl], in_=xk8[:, :, sl])
                nc.sync.dma_start(out=xt_sb[:, 2 * ck:2 * ck + 2, :, :],
                                  in_=xt8[:, 2 * ck:2 * ck + 2, :, :])
                xsq = sqp.tile([P, CT, CH], fp8, tag="xsq")
                for ct, eng in ((0, nc.vector), (1, nc.gpsimd),
                                (2, nc.vector), (3, nc.gpsimd)):
                    eng.tensor_mul(xsq[:, ct, :], xk_sb[:, ct, sl],
                                   xk_sb[:, ct, sl])
                ps_ss = psbs.tile([P, CH], f32, tag="ps")
                for cp in range(2):
                    nc.tensor.matmul(
                        ps_ss[0:1, :],
                        lhsT=ones_f8[:, :, 0:1],
                        rhs=xsq[:, 2 * cp:2 * cp + 2, :],
                        start=(cp == 0), stop=(cp == 1),
                        perf_mode=DR,
                    )
                lnrow = lnp.tile([1, CH], f32, tag="ln")
                nc.scalar.activation(out=lnrow, in_=ps_ss[0:1, :], func=AF.Ln,
                                     scale=1.0 / C)
                lnd = lndp.tile([1, CH], f32, tag="lnd")
                nc.sync.dma_start(out=lnd, in_=lnrow)
                chunk_state[ck] = (lnd, lnrow)

            def emit_chunk_b(ck):
                # transpose landing + per-key-tile exp scale/bias and inv_r
                # columns (PE never waits on these; ACT ops are tiny)
                lnd, lnrow = chunk_state[ck]
                csl = slice(4 * ck, 4 * ck + 4)
                lncol = lnp.tile([P, 4], f32, tag="lncol")
                nc.sync.dma_start(
                    out=lncol,
                    in_=lnd[0:1, :].rearrange("one (j p) -> (one p) j", p=P),
                )
                nc.scalar.activation(out=sccol[:, csl], in_=lncol,
                                     func=AF.Exp, scale=-0.5,
                                     bias=lnc_sb[:, 0:1])
                nc.scalar.activation(out=ir8[:, csl, 0], in_=lncol,
                                     func=AF.Exp, scale=0.5)
                nc.gpsimd.tensor_scalar_mul(bcol[:, csl], lncol, -0.5)

            def emit_chunk_c(ck):
                # T = (M x_q + wkbq) * r_q for the frame's query block
                if ck % 2 != 0:
                    chunk_state.pop(ck)
                    return
                _, lnrow = chunk_state.pop(ck)
                f = ck // 2
                rq = lnp.tile([1, QB], bf16, tag="rq")
                nc.scalar.activation(out=rq, in_=lnrow[0:1, 0:QB],
                                     func=AF.Exp, scale=-0.5)
                ps_rb = psbs.tile([P, CH], f32, tag="ps")
                nc.tensor.matmul(ps_rb[:, 0:QB], lhsT=ones_col_bf,
                                 rhs=rq, start=True, stop=True)
                rbq = rbqp.tile([P, QB], f32, tag="rbq")
                nc.scalar.copy(rbq, ps_rb[:, 0:QB])
                for co in range(CT):
                    pT = psbs.tile([P, CH], f32, tag="ps")
                    for cp in range(2):
                        nc.tensor.matmul(
                            pT[:, 0:QB],
                            lhsT=m_sb[:, 2 * cp:2 * cp + 2,
                                      co * P:(co + 1) * P],
                            rhs=xk_sb[:, 2 * cp:2 * cp + 2,
                                      ck * CH:ck * CH + QB],
                            start=(cp == 0), stop=(cp == 1),
                            perf_mode=DR,
                        )
                    nc.vector.scalar_tensor_tensor(
                        out=T_sb[:, co, f * QB:(f + 1) * QB],
                        in0=pT[:, 0:QB],
                        scalar=bqc_sb[:, co:co + 1],
                        in1=rbq,
                        op0=alu.AluOpType.add,
                        op1=alu.AluOpType.mult,
                    )

            # --------------------------------------------------------------
            # Phase B: block-causal attention, software-pipelined. Scores/exp
            # at key-tile granularity; PV + denominator at key-tile-PAIR
            # granularity (fp8 DR over the pair, x-space lhsT). Pair
            # finalization deferred so it overlaps the next pair.
            # --------------------------------------------------------------
            Q2 = 2 * QB

            tasks = []
            for j in range(F // 2):
                qa = 2 * j
                shared = (2 * j + 1) * (HW // P)
                nkt = shared + HW // P
                for kt in range(nkt):
                    ex = kt >= shared
                    tasks.append(dict(
                        j=j, kt=kt, qa=qa,
                        first=(kt == 0), last=(kt == nkt - 1),
                        qsl=(slice((qa + 1) * QB, (qa + 2) * QB) if ex
                             else slice(qa * QB, qa * QB + Q2)),
                        off=(QB if ex else 0), w=(QB if ex else Q2),
                    ))

            D = 4                      # PV lags scores by D tasks
            # extra lag for the first PV pairs of each frame-pair: the
            # previous pair's po buffer (psbo bufs=1) is released only after
            # its den->rdb->broadcast->fin1 chain (~7us); emitting the first
            # PVs later keeps the in-order PE stream on independent scores
            # instead of blocking on the handoff.
            FIRST_PV_EXTRA = 6
            po_tiles = {}
            et_tiles = {}
            pair_state = {}
            fin1_due = {}              # i -> pair j: broadcast + on-muls
            fin2_due = {}              # i -> pair j: WoWv projection + out

            def emit_scores_exp(i):
                t = tasks[i]
                if t["first"]:
                    po_tiles[t["j"]] = psbo.tile([P, 5, Q2], f32, tag="po",
                                                 name="po%d" % t["j"])
                w = t["w"]
                kt = t["kt"]
                s = i % 2
                if s == 0:
                    et_tiles[i // 2] = etp.tile([P, 2, Q2], fp8, tag="et",
                                                name="et%d" % (i // 2))
                et = et_tiles[i // 2]
                ps = psbs.tile([P, Q2], f32, tag="ps")
                for cp in range(2):
                    nc.tensor.matmul(
                        ps[:, :w],
                        lhsT=xk_sb[:, 2 * cp:2 * cp + 2,
                                   kt * P:(kt + 1) * P],
                        rhs=T_sb[:, 2 * cp:2 * cp + 2, t["qsl"]],
                        start=(cp == 0), stop=(cp == 1),
                        perf_mode=DR,
                    )
                nc.scalar.activation(out=et[:, s, :w], in_=ps[:, :w],
                                     func=AF.Exp,
                                     scale=sccol[:, kt:kt + 1],
                                     bias=bcol[:, kt:kt + 1])

            def emit_pv(i, cur_i):
                # i is the ODD task of the pair (i-1, i)
                t = tasks[i]
                et = et_tiles.pop(i // 2)
                po = po_tiles[t["j"]]
                w, off = t["w"], t["off"]
                kp = t["kt"] // 2
                first = t["kt"] == 1
                for ct in range(CT):
                    nc.tensor.matmul(
                        po[:, ct, off:],
                        lhsT=xt_sb[:, kp, :, ct * P:(ct + 1) * P],
                        rhs=et[:, :, :w],
                        start=first, stop=t["last"],
                        perf_mode=DR,
                        skip_group_check=True,
                    )
                nc.tensor.matmul(
                    po[0:1, 4, off:],
                    lhsT=ir8[:, 2 * kp:2 * kp + 2, 0:1],
                    rhs=et[:, :, :w],
                    start=first, stop=t["last"], perf_mode=DR,
                    skip_group_check=True,
                )
                if t["last"]:
                    rd = smp.tile([1, Q2], f32, tag="rd")
                    nc.scalar.activation(out=rd, in_=po[0:1, 4, :], func=AF.Ln)
                    rdb = smp.tile([1, Q2], bf16, tag="rdb")
                    nc.scalar.activation(out=rdb, in_=rd, func=AF.Exp,
                                         scale=-1.0)
                    pair_state[t["j"]] = rdb
                    fin1_due[cur_i + 1] = t["j"]
                    fin2_due[cur_i + 5] = t["j"]

            def emit_fin1(j):
                po = po_tiles[j]
                rdb = pair_state[j]
                rb2_ps = psbs.tile([P, Q2], f32, tag="ps")
                nc.tensor.matmul(rb2_ps, lhsT=ones_col_bf, rhs=rdb,
                                 start=True, stop=True)
                rb2 = smp.tile([P, Q2], f32, tag="rb2")
                nc.scalar.copy(rb2, rb2_ps)
                on = smp.tile([P, CT, Q2], fp8, tag="on")
                for ct in range(CT):
                    nc.vector.tensor_mul(on[:, ct, :], po[:, ct, :], rb2)
                pair_state[j] = on

            def emit_fin2(j):
                on = pair_state.pop(j)
                po_tiles.pop(j)
                qa = 2 * j
                for co in range(CT):
                    pf = psbs.tile([P, Q2], f32, tag="ps")
                    for cp in range(2):
                        nc.tensor.matmul(
                            pf,
                            lhsT=wow_sb[:, 2 * cp:2 * cp + 2,
                                        co * P:(co + 1) * P],
                            rhs=on[:, 2 * cp:2 * cp + 2, :],
                            start=(cp == 0), stop=(cp == 1),
                            perf_mode=DR,
                        )
                    xres_t = xrp.tile([P, Q2], f32, tag="xres")
                    nc.sync.dma_start(out=xres_t,
                                      in_=xqres[:, co, qa:qa + 2, :])
                    ot = outp.tile([P, Q2], f32, tag="ot")
                    nc.vector.scalar_tensor_tensor(
                        out=ot, in0=pf, scalar=1.0 / WS_O, in1=xres_t,
                        op0=alu.AluOpType.mult, op1=alu.AluOpType.add,
                    )
                    nc.sync.dma_start(
                        out=out[:, co, qa:qa + 2, :],
                        in_=ot[:, :].rearrange("p (f t) -> p f t", t=QB),
                    )

            # chunk stage deadlines: pair j (starting at task S_j in
            # {0,16,48,96}) reads chunks <= 4j+3, so each chunk's 3 stages
            # are spread over the earlier task stream (stage offsets keep
            # the PE from waiting on fresh ACT/DVE work).
            stage_a = {0: 4, 1: 5, 2: 6, 3: 7,
                       10: 8, 14: 9, 18: 10, 22: 11,
                       36: 12, 42: 13, 48: 14, 54: 15}
            stage_b = {i + 2: c for i, c in stage_a.items()}
            stage_c = {i + 4: c for i, c in stage_a.items()}

            # startup: chunk 0/2 (T for frames 0-1) pipelined ahead of the
            # later chunks so the first scores aren't queued behind 4 chunks
            # of squares on the Pool/DVE queues
            for step in (lambda: emit_chunk_a(0), lambda: emit_chunk_a(1),
                         lambda: emit_chunk_b(0), lambda: emit_chunk_c(0),
                         lambda: emit_chunk_a(2), lambda: emit_chunk_b(1),
                         lambda: emit_chunk_c(1), lambda: emit_chunk_a(3),
                         lambda: emit_chunk_b(2), lambda: emit_chunk_c(2),
                         lambda: emit_chunk_b(3), lambda: emit_chunk_c(3)):
                step()

            n = len(tasks)
            pv_sched = {}
            for ip in range(1, n, 2):
                t = tasks[ip]
                lag = D + max(0, FIRST_PV_EXTRA - (t["kt"] - 1))
                pv_sched.setdefault(ip + lag, []).append(ip)

            for i in range(n + D + FIRST_PV_EXTRA + 3):
                if i in stage_a:
                    emit_chunk_a(stage_a[i])
                if i in stage_b:
                    emit_chunk_b(stage_b[i])
                if i in stage_c:
                    emit_chunk_c(stage_c[i])
                if i < n:
                    emit_scores_exp(i)
                if i in fin1_due:
                    emit_fin1(fin1_due.pop(i))
                if i in fin2_due:
                    emit_fin2(fin2_due.pop(i))
                for ip in pv_sched.pop(i, ()):
                    emit_pv(ip, i)


_NC = None


def _get_nc():
    global _NC
    if _NC is None:
        _install_fix()
        _NC = _build_nc()
    return _NC


def _to_pco(a):
    """[C, ...] -> [P, CT, ...] with channel c = ct*128 + p."""
    return np.ascontiguousarray(
        a.reshape(CT, P, *a.shape[1:]).swapaxes(0, 1)
    )


def prepare_in_maps(inputs):
    x = np.asarray(inputs["x"], dtype=np.float32)
    gamma = np.asarray(inputs["gamma"], dtype=np.float32).reshape(C)
    wq, wk, wv, wo = (np.asarray(inputs[k], dtype=np.float32)
                      for k in ("wq", "wk", "wv", "wo"))
    bq, bk, bv, bo = (np.asarray(inputs[k], dtype=np.float32)
                      for k in ("bq", "bk", "bv", "bo"))

    # composed weights (gamma folds into the input-channel side everywhere)
    wk_eff = wk * gamma[None, :]
    wq_eff = wq * gamma[None, :]
    M_eff = wk_eff.T @ wq_eff                  # scores = x^T M x
    wkbq = wk_eff.T @ bq                       # per-channel T bias
    wow_eff = (wo @ wv) * gamma[None, :]       # out = (Wo Wv) o~
    res_bias = bo + wo @ bv                    # exact: softmax rows sum to 1

    m_t = _to_pco(np.ascontiguousarray(M_eff.T * WS_M))
    m_t = np.clip(m_t, -240.0, 240.0).astype(ml_dtypes.float8_e4m3)
    wow_t = _to_pco(np.ascontiguousarray(wow_eff.T * WS_O))
    wow_t = np.clip(wow_t, -240.0, 240.0).astype(ml_dtypes.float8_e4m3)
    bq_col = _to_pco(np.ascontiguousarray(wkbq * WS_M))  # [P, CT]

    xf = x.reshape(B, C, F, HW)
    # frame-internal permutation: core's query block first (attention is
    # order-invariant within a frame, so keys may be permuted per core)
    perms = []
    for ch in range(4):
        qidx = np.arange(ch * QB, (ch + 1) * QB)
        rest = np.array([i for i in range(HW)
                         if not (ch * QB <= i < (ch + 1) * QB)])
        perms.append(np.concatenate([qidx, rest]))
    in_maps = []
    for core in range(N_CORES):
        b = core // 4
        ch = core % 4
        xp = np.ascontiguousarray(
            xf[b][:, :, perms[ch]].reshape(C, S))                 # [C, S]
        xk8 = _to_pco(xp).astype(ml_dtypes.float8_e4m3)
        xt8 = np.ascontiguousarray(
            xp.T.reshape(KP, 2, P, C).transpose(2, 0, 1, 3)
        ).astype(ml_dtypes.float8_e4m3)                           # [P,KP,2,C]
        xq_c = xf[b, :, :, ch * QB:(ch + 1) * QB]                 # [C, F, QB]
        xqres = _to_pco(
            np.ascontiguousarray(xq_c + res_bias[:, None, None])
        )                                                         # [P,CT,F,QB]
        in_maps.append({
            "xk8": xk8, "xt8": xt8, "xqres": xqres,
            "m_t": m_t, "wow_t": wow_t, "bq_col": bq_col,
        })
    return in_maps


def kernel(x, gamma, wq, bq, wk, bk, wv, bv, wo, bo):
    in_maps = prepare_in_maps(dict(x=x, gamma=gamma, wq=wq, bq=bq, wk=wk,
                                   bk=bk, wv=wv, bv=bv, wo=wo, bo=bo))
    nc = _get_nc()
    res = run_bass_kernel_spmd(nc, in_maps, core_ids=list(range(N_CORES)))

    out = np.empty((B, C, F, HW), dtype=np.float32)
    for core in range(N_CORES):
        b = core // 4
        ch = core % 4
        o = res.results[core]["out"]              # [P, CT, F, QB]
        o = o.swapaxes(0, 1).reshape(C, F, QB)    # [C, F, QB]
        out[b, :, :, ch * QB:(ch + 1) * QB] = o
    return out.reshape(B, C, F, H, W)


# revision 54
# speedup vs baseline: 1.1466x; 1.1466x over previous
"""Trainium2 Bass kernel for nn_AttnBlock (block-causal single-head attention
over video tokens, with RMS-norm and 1x1-conv q/k/v/out projections).

Shapes: x [2, 512, 8, 32, 32] -> S = 8*1024 = 8192 tokens per batch,
block-causal over frames (1024 tokens per frame).

Sharding: core = 4*b + ch handles batch b and the ch-th 256-query chunk of
EVERY frame -> all 8 cores run an identical instruction stream (SPMD) with
perfectly balanced block-causal attention work.

Key algebraic restructure (vs a direct port of the reference):
  * The 1x1 convs commute with attention, so the K and V projections are
    ELIMINATED by composing weights on the host:
      scores = K^T Q = x^T (Wk^T Wq) x       -> one matrix M, applied only
                                                to the 2048 query tokens/core
      P V    = Wv (P x)                      -> attention runs in x-space
      Wo(Wv (P x)) = (Wo Wv)(P x)            -> one composed output proj
  * The RMS-norm scale r[t] = sqrt(C)/||x_t|| commutes past the linear maps:
      - r_q is folded into the T = M x_q build (row broadcast, tiny)
      - r_k enters via the ACT exp's per-partition scale AND bias APs:
          et = r_k * exp(c * r_k * s_raw)    (bias = ln r_k)
        so PV over raw x^T tiles yields o~ = sum_k p_k r_k x_k directly
      - the denominator matmul uses lhsT = fp8(1/r_k) so den = sum_k exp
  * bk drops exactly (softmax per-query shift invariance); bq folds into the
    T bias (exact for bq=0, first-order otherwise); bv,bo fold into the
    residual (exact, since softmax rows sum to 1).

fp8: x itself is shipped as fp8e4 in BOTH layouts (channel-partition for
score lhsT / T rhs, token-partition for PV lhsT). M is fp8 scaled x128;
WoWv stays bf16. All big matmuls are fp8 DoubleRow (K=256 per column).
"""

import numpy as np
import ml_dtypes
from contextlib import ExitStack

# ---------------------------------------------------------------------------
# Walrus workaround: this container's walrus build accepts at most ONE sync
# wait command per instruction. Split excess waits onto same-engine NOPs
# (waits execute strictly earlier -> safe), including the Tile exit drain.
# ---------------------------------------------------------------------------
import bass_rust
import concourse.bass as bass
import concourse.mybir as mybir
import concourse.tile as tile
from concourse.vector_clock import ScopedClock
from concourse.bass_utils import run_bass_kernel_spmd

_MAX_WAITS = 1
_orig_lower = tile.TileContext._lower_ordered_insts


def _split_waits(nc, ordered):
    for bb, insts in ordered.items():
        out = []
        for inst in insts:
            si = inst.sync_info
            waits = list(si.on_wait) if si is not None and si.on_wait else []
            if (
                len(waits) > _MAX_WAITS
                and inst.engine is not None
                and inst.engine != mybir.EngineType.Unassigned
            ):
                for w in waits[:-_MAX_WAITS]:
                    out.append(
                        mybir.InstNoOp(
                            name=nc.get_next_instruction_name(),
                            engine=inst.engine,
                            bass_nofuse=True,
                            sync_info=mybir.SyncInfo(on_wait=[w], on_update=[]),
                        )
                    )
                si.on_wait = waits[-_MAX_WAITS:]
            out.append(inst)
        ordered[bb] = out


def _patched_lower(self, ordered):
    _split_waits(self.nc, ordered)
    return _orig_lower(self, ordered)


def _patched_drain_and_barrier(self, tick_clock, wait_clock):
    nc = self.nc
    drain_inst = nc.sync.drain()
    wait_clock.add_sem_waits(
        drain_inst.ins, ScopedClock({None: tick_clock.global_clock})
    )
    si = drain_inst.ins.sync_info
    waits = list(si.on_wait or []) if si is not None else []
    if len(waits) > _MAX_WAITS:
        si.on_wait = waits[:_MAX_WAITS]
        # distribute the remaining waits across all engine queues so the
        # exit drain's wait-processing runs in parallel instead of as one
        # serial NOP chain on the sync sequencer
        engines = [nc.sync, nc.vector, nc.scalar, nc.tensor, nc.gpsimd]
        for k, i in enumerate(range(_MAX_WAITS, len(waits), _MAX_WAITS)):
            n = engines[k % len(engines)].nop(nofuse=True)
            n.ins.sync_info = bass_rust.SyncInfo(
                on_wait=waits[i:i + _MAX_WAITS], on_update=[]
            )
    nc.all_engine_barrier()
    assert self.sems is not None
    popped = nc._tile_sem_poison_stack.pop()
    assert popped is self._sem_poison
    nc.clear_and_free_semaphores(list(self.sems.allocated().values()))
    nc.all_engine_barrier()


def _install_fix():
    tile.TileContext._lower_ordered_insts = _patched_lower
    tile.TileContext._drain_and_barrier = _patched_drain_and_barrier


# ---------------------------------------------------------------------------
# Problem constants (hardcoded per contract)
# ---------------------------------------------------------------------------
B, C, F, H, W = 2, 512, 8, 32, 32
HW = H * W            # 1024 tokens per frame
S = F * HW            # 8192 tokens per batch
P = 128
CT = C // P           # 4 channel tiles
QB = 256              # query block per frame per core
TQ = F * QB           # 2048 queries per core
CH = 512              # phase-A token chunk
NCH = S // CH         # 16
NKT = S // P          # 64 key tiles of 128
KP = NKT // 2         # 32 key-tile pairs
N_CORES = 8

WS_M = 128.0          # host-side fp8 range scale on M = Wk^T Wq
WS_O = 128.0          # host-side fp8 range scale on WoWv

f32 = mybir.dt.float32
bf16 = mybir.dt.bfloat16
fp8 = mybir.dt.float8e4
AF = mybir.ActivationFunctionType
DR = mybir.MatmulPerfMode.DoubleRow
ALU = None


def _build_nc(reps=1):
    nc = bass.Bass("TRN2")

    xk8 = nc.dram_tensor("xk8", [P, CT, S], fp8, kind="ExternalInput")
    xt8 = nc.dram_tensor("xt8", [P, KP, 2, C], fp8, kind="ExternalInput")
    xqres = nc.dram_tensor("xqres", [P, CT, F, QB], f32, kind="ExternalInput")
    m_t = nc.dram_tensor("m_t", [P, CT, C], fp8, kind="ExternalInput")
    wow_t = nc.dram_tensor("wow_t", [P, CT, C], fp8, kind="ExternalInput")
    bq_col = nc.dram_tensor("bq_col", [P, CT], f32, kind="ExternalInput")
    out = nc.dram_tensor("out", [P, CT, F, QB], f32, kind="ExternalOutput")

    with tile.TileContext(nc) as tc:
        for _ in range(reps):
            _emit_once(nc, tc, xk8, xt8, xqres, m_t, wow_t, bq_col, out)
    return nc


def _emit_once(nc, tc, xk8, xt8, xqres, m_t, wow_t, bq_col, out):
    import concourse.alu_op_type as alu
    with ExitStack() as ctx:
        big = ctx.enter_context(tc.tile_pool(name="big", bufs=1))
        xk_sb = big.tile([P, CT, S], fp8)
        xt_sb = big.tile([P, KP, 2, C], fp8)
        T_sb = big.tile([P, CT, TQ], fp8)
        sccol = big.tile([P, NKT], f32)     # exp per-partition scale c*r_k
        bcol = big.tile([P, NKT], f32)      # exp per-partition bias ln(r_k)
        # 1/r_k denominator lhsT, padded to stride 16 so the DR LDWEIGHTS
        # AP matches the encodable pattern (row stride >= 16)
        ir8 = big.tile([P, NKT, 16], fp8)

        const = ctx.enter_context(tc.tile_pool(name="const", bufs=1))
        ones_f8 = const.tile([P, 2, 16], fp8)
        nc.vector.memset(ones_f8, 1.0)
        ones_col_bf = const.tile([1, P], bf16)
        nc.vector.memset(ones_col_bf, 1.0)
        lnc_sb = const.tile([P, 1], f32)
        m_sb = const.tile([P, CT, C], fp8)
        nc.sync.dma_start(out=m_sb, in_=m_t[:, :, :])
        wow_sb = const.tile([P, CT, C], fp8)
        nc.sync.dma_start(out=wow_sb, in_=wow_t[:, :, :])
        bqc_sb = const.tile([P, CT], f32)
        nc.sync.dma_start(out=bqc_sb, in_=bq_col[:, :])

        # exp scale constant: exp arg = (r_k / (sqrt(C)*WS_M)) * ps
        c_const = 1.0 / (float(np.sqrt(C)) * WS_M)
        nc.vector.memset(lnc_sb, float(np.log(c_const)))

        with (
            tc.tile_pool(name="sq", bufs=3) as sqp,
            tc.tile_pool(name="lnr", bufs=5) as lnp,
            tc.tile_pool(name="lnd", bufs=5, space="DRAM") as lndp,
            tc.tile_pool(name="rbq", bufs=2) as rbqp,
            tc.tile_pool(name="xres", bufs=2) as xrp,
            tc.tile_pool(name="etp", bufs=8) as etp,
            tc.tile_pool(name="smp", bufs=2) as smp,
            tc.tile_pool(name="outp", bufs=3) as outp,
            tc.tile_pool(name="ps", bufs=3, space="PSUM") as psbs,
            tc.tile_pool(name="psbo", bufs=1, space="PSUM") as psbo,
        ):
            # --------------------------------------------------------------
            # Phase A chunk: load x chunk (both layouts), sumsq -> ln ->
            # (transpose to columns) -> exp scale/bias/inv_r; on even chunks
            # build T = (M x_q + wkbq) * r_q for the frame's query block.
            # --------------------------------------------------------------
            chunk_state = {}

            def emit_chunk_a(ck):
                # DMA both layouts, square (split Pool/DVE), row-sumsq
                # matmuls, ln, DRAM bounce of the ln row
                sl = slice(ck * CH, (ck + 1) * CH)
                nc.sync.dma_start(out=xk_sb[:, :, sl], in_=xk8[:, :, sl])
                nc.sync.dma_start(out=xt_sb[:, 2 * ck:2 * ck + 2, :, :],
                                  in_=xt8[:, 2 * ck:2 * ck + 2, :, :])
                xsq = sqp.tile([P, CT, CH], fp8, tag="xsq")
                nc.gpsimd.tensor_mul(xsq[:, 0:2, :], xk_sb[:, 0:2, sl],
                                     xk_sb[:, 0:2, sl])
                nc.vector.tensor_mul(xsq[:, 2:4, :], xk_sb[:, 2:4, sl],
                                     xk_sb[:, 2:4, sl])
                ps_ss = psbs.tile([P, CH], f32, tag="ps")
                for cp in range(2):
                    nc.tensor.matmul(
                        ps_ss[0:1, :],
                        lhsT=ones_f8[:, :, 0:1],
                        rhs=xsq[:, 2 * cp:2 * cp + 2, :],
                        start=(cp == 0), stop=(cp == 1),
                        perf_mode=DR,
                    )
                lnrow = lnp.tile([1, CH], f32, tag="ln")
                nc.scalar.activation(out=lnrow, in_=ps_ss[0:1, :], func=AF.Ln,
                                     scale=1.0 / C)
                lnd = lndp.tile([1, CH], f32, tag="lnd")
                nc.sync.dma_start(out=lnd, in_=lnrow)
                chunk_state[ck] = (lnd, lnrow)

            def emit_chunk_b(ck):
                # transpose landing + per-key-tile exp scale/bias and inv_r
                # columns (PE never waits on these; ACT ops are tiny)
                lnd, lnrow = chunk_state[ck]
                csl = slice(4 * ck, 4 * ck + 4)
                lncol = lnp.tile([P, 4], f32, tag="lncol")
                nc.sync.dma_start(
                    out=lncol,
                    in_=lnd[0:1, :].rearrange("one (j p) -> (one p) j", p=P),
                )
                nc.scalar.activation(out=sccol[:, csl], in_=lncol,
                                     func=AF.Exp, scale=-0.5,
                                     bias=lnc_sb[:, 0:1])
                nc.scalar.activation(out=ir8[:, csl, 0], in_=lncol,
                                     func=AF.Exp, scale=0.5)
                nc.gpsimd.tensor_scalar_mul(bcol[:, csl], lncol, -0.5)

            def emit_chunk_c(ck):
                # T = (M x_q + wkbq) * r_q for the frame's query block
                if ck % 2 != 0:
                    chunk_state.pop(ck)
                    return
                _, lnrow = chunk_state.pop(ck)
                f = ck // 2
                rq = lnp.tile([1, QB], bf16, tag="rq")
                nc.scalar.activation(out=rq, in_=lnrow[0:1, 0:QB],
                                     func=AF.Exp, scale=-0.5)
                ps_rb = psbs.tile([P, CH], f32, tag="ps")
                nc.tensor.matmul(ps_rb[:, 0:QB], lhsT=ones_col_bf,
                                 rhs=rq, start=True, stop=True)
                rbq = rbqp.tile([P, QB], f32, tag="rbq")
                nc.scalar.copy(rbq, ps_rb[:, 0:QB])
                for co in range(CT):
                    pT = psbs.tile([P, CH], f32, tag="ps")
                    for cp in range(2):
                        nc.tensor.matmul(
                            pT[:, 0:QB],
                            lhsT=m_sb[:, 2 * cp:2 * cp + 2,
                                      co * P:(co + 1) * P],
                            rhs=xk_sb[:, 2 * cp:2 * cp + 2,
                                      ck * CH:ck * CH + QB],
                            start=(cp == 0), stop=(cp == 1),
                            perf_mode=DR,
                        )
                    nc.vector.scalar_tensor_tensor(
                        out=T_sb[:, co, f * QB:(f + 1) * QB],
                        in0=pT[:, 0:QB],
                        scalar=bqc_sb[:, co:co + 1],
                        in1=rbq,
                        op0=alu.AluOpType.add,
                        op1=alu.AluOpType.mult,
                    )

            # --------------------------------------------------------------
            # Phase B: block-causal attention, software-pipelined. Scores/exp
            # at key-tile granularity; PV + denominator at key-tile-PAIR
            # granularity (fp8 DR over the pair, x-space lhsT). Pair
            # finalization deferred so it overlaps the next pair.
            # --------------------------------------------------------------
            Q2 = 2 * QB

            tasks = []
            for j in range(F // 2):
                qa = 2 * j
                shared = (2 * j + 1) * (HW // P)
                nkt = shared + HW // P
                for kt in range(nkt):
                    ex = kt >= shared
                    tasks.append(dict(
                        j=j, kt=kt, qa=qa,
                        first=(kt == 0), last=(kt == nkt - 1),
                        qsl=(slice((qa + 1) * QB, (qa + 2) * QB) if ex
                             else slice(qa * QB, qa * QB + Q2)),
                        off=(QB if ex else 0), w=(QB if ex else Q2),
                    ))

            D = 4                      # PV lags scores by D tasks
            po_tiles = {}
            et_tiles = {}
            pair_state = {}
            fin1_due = {}              # i -> pair j: broadcast + on-muls
            fin2_due = {}              # i -> pair j: WoWv projection + out

            def emit_scores_exp(i):
                t = tasks[i]
                if t["first"]:
                    po_tiles[t["j"]] = psbo.tile([P, 5, Q2], f32, tag="po",
                                                 name="po%d" % t["j"])
                w = t["w"]
                kt = t["kt"]
                s = i % 2
                if s == 0:
                    et_tiles[i // 2] = etp.tile([P, 2, Q2], fp8, tag="et",
                                                name="et%d" % (i // 2))
                et = et_tiles[i // 2]
                ps = psbs.tile([P, Q2], f32, tag="ps")
                for cp in range(2):
                    nc.tensor.matmul(
                        ps[:, :w],
                        lhsT=xk_sb[:, 2 * cp:2 * cp + 2,
                                   kt * P:(kt + 1) * P],
                        rhs=T_sb[:, 2 * cp:2 * cp + 2, t["qsl"]],
                        start=(cp == 0), stop=(cp == 1),
                        perf_mode=DR,
                    )
                nc.scalar.activation(out=et[:, s, :w], in_=ps[:, :w],
                                     func=AF.Exp,
                                     scale=sccol[:, kt:kt + 1],
                                     bias=bcol[:, kt:kt + 1])

            def emit_pv(i, cur_i):
                # i is the ODD task of the pair (i-1, i)
                t = tasks[i]
                et = et_tiles.pop(i // 2)
                po = po_tiles[t["j"]]
                w, off = t["w"], t["off"]
                kp = t["kt"] // 2
                first = t["kt"] == 1
                for ct in range(CT):
                    nc.tensor.matmul(
                        po[:, ct, off:],
                        lhsT=xt_sb[:, kp, :, ct * P:(ct + 1) * P],
                        rhs=et[:, :, :w],
                        start=first, stop=t["last"],
                        perf_mode=DR,
                        skip_group_check=True,
                    )
                nc.tensor.matmul(
                    po[0:1, 4, off:],
                    lhsT=ir8[:, 2 * kp:2 * kp + 2, 0:1],
                    rhs=et[:, :, :w],
                    start=first, stop=t["last"], perf_mode=DR,
                    skip_group_check=True,
                )
                if t["last"]:
                    rd = smp.tile([1, Q2], f32, tag="rd")
                    nc.scalar.activation(out=rd, in_=po[0:1, 4, :], func=AF.Ln)
                    rdb = smp.tile([1, Q2], bf16, tag="rdb")
                    nc.scalar.activation(out=rdb, in_=rd, func=AF.Exp,
                                         scale=-1.0)
                    pair_state[t["j"]] = rdb
                    fin1_due[cur_i + 1] = t["j"]
                    fin2_due[cur_i + 4] = t["j"]

            def emit_fin1(j):
                po = po_tiles[j]
                rdb = pair_state[j]
                rb2_ps = psbs.tile([P, Q2], f32, tag="ps")
                nc.tensor.matmul(rb2_ps, lhsT=ones_col_bf, rhs=rdb,
                                 start=True, stop=True)
                rb2 = smp.tile([P, Q2], f32, tag="rb2")
                nc.scalar.copy(rb2, rb2_ps)
                on = smp.tile([P, CT, Q2], fp8, tag="on")
                for ct in range(CT):
                    nc.vector.tensor_mul(on[:, ct, :], po[:, ct, :], rb2)
                pair_state[j] = on

            def emit_fin2(j):
                on = pair_state.pop(j)
                po_tiles.pop(j)
                qa = 2 * j
                for co in range(CT):
                    pf = psbs.tile([P, Q2], f32, tag="ps")
                    for cp in range(2):
                        nc.tensor.matmul(
                            pf,
                            lhsT=wow_sb[:, 2 * cp:2 * cp + 2,
                                        co * P:(co + 1) * P],
                            rhs=on[:, 2 * cp:2 * cp + 2, :],
                            start=(cp == 0), stop=(cp == 1),
                            perf_mode=DR,
                        )
                    xres_t = xrp.tile([P, Q2], f32, tag="xres")
                    nc.sync.dma_start(out=xres_t,
                                      in_=xqres[:, co, qa:qa + 2, :])
                    ot = outp.tile([P, Q2], f32, tag="ot")
                    nc.vector.scalar_tensor_tensor(
                        out=ot, in0=pf, scalar=1.0 / WS_O, in1=xres_t,
                        op0=alu.AluOpType.mult, op1=alu.AluOpType.add,
                    )
                    nc.sync.dma_start(
                        out=out[:, co, qa:qa + 2, :],
                        in_=ot[:, :].rearrange("p (f t) -> p f t", t=QB),
                    )

            # chunk stage deadlines: pair j (starting at task S_j in
            # {0,16,48,96}) reads chunks <= 4j+3, so each chunk's 3 stages
            # are spread over the earlier task stream (stage offsets keep
            # the PE from waiting on fresh ACT/DVE work).
            stage_a = {0: 4, 1: 5, 2: 6, 3: 7,
                       12: 8, 16: 9, 20: 10, 24: 11,
                       40: 12, 46: 13, 52: 14, 58: 15}
            stage_b = {i + 4: c for i, c in stage_a.items()}
            stage_c = {i + 8: c for i, c in stage_a.items()}

            for ck in range(4):
                emit_chunk_a(ck)
            for ck in range(4):
                emit_chunk_b(ck)
            for ck in range(4):
                emit_chunk_c(ck)

            n = len(tasks)
            # pairs j>=1: delay the first PV emissions so the in-order PE
            # stream keeps running independent scores while the previous
            # pair's den->rdb->broadcast->fin1 chain releases the po buffer
            # (psbo bufs=1). Slots converge, preserving PSUM group order.
            EXTRA = 6
            pv_sched = {}
            for ip in range(1, n, 2):
                t = tasks[ip]
                lag = D
                if t["j"] > 0:
                    lag += max(0, EXTRA - (t["kt"] - 1))
                pv_sched.setdefault(ip + lag, []).append(ip)

            for i in range(n + D + EXTRA + 3):
                if i in stage_a:
                    emit_chunk_a(stage_a[i])
                if i in stage_b:
                    emit_chunk_b(stage_b[i])
                if i in stage_c:
                    emit_chunk_c(stage_c[i])
                if i < n:
                    emit_scores_exp(i)
                if i in fin1_due:
                    emit_fin1(fin1_due.pop(i))
                if i in fin2_due:
                    emit_fin2(fin2_due.pop(i))
                for ip in pv_sched.pop(i, ()):
                    emit_pv(ip, i)


_NC = None


def _get_nc():
    global _NC
    if _NC is None:
        _install_fix()
        _NC = _build_nc()
    return _NC


def _to_pco(a):
    """[C, ...] -> [P, CT, ...] with channel c = ct*128 + p."""
    return np.ascontiguousarray(
        a.reshape(CT, P, *a.shape[1:]).swapaxes(0, 1)
    )


def prepare_in_maps(inputs):
    x = np.asarray(inputs["x"], dtype=np.float32)
    gamma = np.asarray(inputs["gamma"], dtype=np.float32).reshape(C)
    wq, wk, wv, wo = (np.asarray(inputs[k], dtype=np.float32)
                      for k in ("wq", "wk", "wv", "wo"))
    bq, bk, bv, bo = (np.asarray(inputs[k], dtype=np.float32)
                      for k in ("bq", "bk", "bv", "bo"))

    # composed weights (gamma folds into the input-channel side everywhere)
    wk_eff = wk * gamma[None, :]
    wq_eff = wq * gamma[None, :]
    M_eff = wk_eff.T @ wq_eff                  # scores = x^T M x
    wkbq = wk_eff.T @ bq                       # per-channel T bias
    wow_eff = (wo @ wv) * gamma[None, :]       # out = (Wo Wv) o~
    res_bias = bo + wo @ bv                    # exact: softmax rows sum to 1

    m_t = _to_pco(np.ascontiguousarray(M_eff.T * WS_M))
    m_t = np.clip(m_t, -240.0, 240.0).astype(ml_dtypes.float8_e4m3)
    wow_t = _to_pco(np.ascontiguousarray(wow_eff.T * WS_O))
    wow_t = np.clip(wow_t, -240.0, 240.0).astype(ml_dtypes.float8_e4m3)
    bq_col = _to_pco(np.ascontiguousarray(wkbq * WS_M))  # [P, CT]

    xf = x.reshape(B, C, F, HW)
    # frame-internal permutation: core's query block first (attention is
    # order-invariant within a frame, so keys may be permuted per core)
    perms = []
    for ch in range(4):
        qidx = np.arange(ch * QB, (ch + 1) * QB)
        rest = np.array([i for i in range(HW)
                         if not (ch * QB <= i < (ch + 1) * QB)])
        perms.append(np.concatenate([qidx, rest]))
    in_maps = []
    for core in range(N_CORES):
        b = core // 4
        ch = core % 4
        xp = np.ascontiguousarray(
            xf[b][:, :, perms[ch]].reshape(C, S))                 # [C, S]
        xk8 = _to_pco(xp).astype(ml_dtypes.float8_e4m3)
        xt8 = np.ascontiguousarray(
            xp.T.reshape(KP, 2, P, C).transpose(2, 0, 1, 3)
        ).astype(ml_dtypes.float8_e4m3)                           # [P,KP,2,C]
        xq_c = xf[b, :, :, ch * QB:(ch + 1) * QB]                 # [C, F, QB]
        xqres = _to_pco(
            np.ascontiguousarray(xq_c + res_bias[:, None, None])
        )                                                         # [P,CT,F,QB]
        in_maps.append({
            "xk8": xk8, "xt8": xt8, "xqres": xqres,
            "m_t": m_t, "wow_t": wow_t, "bq_col": bq_col,
        })
    return in_maps


def kernel(x, gamma, wq, bq, wk, bk, wv, bv, wo, bo):
    in_maps = prepare_in_maps(dict(x=x, gamma=gamma, wq=wq, bq=bq, wk=wk,
                                   bk=bk, wv=wv, bv=bv, wo=wo, bo=bo))
    nc = _get_nc()
    res = run_bass_kernel_spmd(nc, in_maps, core_ids=list(range(N_CORES)))

    out = np.empty((B, C, F, HW), dtype=np.float32)
    for core in range(N_CORES):
        b = core // 4
        ch = core % 4
        o = res.results[core]["out"]              # [P, CT, F, QB]
        o = o.swapaxes(0, 1).reshape(C, F, QB)    # [C, F, QB]
        out[b, :, :, ch * QB:(ch + 1) * QB] = o
    return out.reshape(B, C, F, H, W)
